# revision 34
# baseline (speedup 1.0000x reference)
"""Multi-head self-attention TRN2 kernel (16 heads, D=1024, x:[2,2048,1024]).

Sharding: 8 cores = 2 (batch) x 4 (head groups of 4 heads).  Host sums the
4 partials per batch and adds bo (the tensor-parallel all-reduce).

Per-core structure (final):
  inputs x/wq/wk/wv DMA'd as bf16 (halves the startup DMA traffic), wo f32r;
  the 32 xT chunks are spread across the SP/Pool/ACT DMA queues in first-use
  order because each queue's sequencer holds a DMA for ~1.1-1.26us.
  qT/kT = (x_b @ wq/wk + b)^T head-dim-major             [256, 2048] f32r
  v     = x_b @ wv + bv, token-major, ones-augmented      [2048,4,65] bf16
  scores: kT-stationary matmuls [k=128chunk, q=512] f32r PSUM
  exp via ACT (scale=1/8, zero-bias AP, no max subtraction) -> bf16 SBUF
  AV q-stationary: lhsT=exp[128k,128q], rhs=v[128k,65] bf16, out [128q,65]
    (65 moving rows/matmul = half the PE-row cost of a d-major AV),
    accumulated with start=False onto DVE-zeroed PSUM (one bank holds the
    four per-qsub regions; the HW start flag would zero the whole 2KB bank)
  normalize: DVE reciprocal of sums column + broadcast multiply -> opair
  PE transpose (vs identity) -> oT, DVE copies PSUM->SBUF
  out = oT^T @ wo per 128-token chunk, bf16 out (host adds bo, upcasts)

Pipeline: 8 windows = (pair0 n0..n3, pair1 n0..n3).  Window w computes
scores+exp for its own 512-q chunk while the PE drains the AV matmuls of
window w-1 (whose exp tiles are all ready, so AV never head-of-line blocks
the in-order PE queue behind an unsatisfied semaphore).  Projections for
k/q/v interleave as PE filler in the early windows where ACT is warming
up; windows 5..7 drain at two g-groups per slot, normalize mid-window, and
run transposes + output projection in-window to keep the tail short.

TimelineSim: 167.6us (baseline v1: 224.1us).  PE busy ~145us (the floor:
projections 41 + scores 54.6 + AV 27.7 + oproj 13.7 + transposes/warmup),
ACT exp 133us, both near-saturated mid-stream.
"""

import os
import sys
from contextlib import ExitStack

import numpy as np

for _p in ("/opt/trn_rl_repo", os.path.expanduser("~/.axon_site/_ro/trn_rl_repo")):
    if os.path.isdir(_p) and _p not in sys.path:
        sys.path.insert(0, _p)

import concourse.bass as bass  # noqa: E402
import concourse.mybir as mybir  # noqa: E402
import concourse.tile as tile  # noqa: E402
from concourse import bacc  # noqa: E402
from concourse.bass_utils import run_bass_kernel_spmd  # noqa: E402

f32 = mybir.dt.float32
f32r = mybir.dt.float32r
bf16 = mybir.dt.bfloat16
P = 128


def build_core_program(D=1024, TOK=2048, NH=4, num_devices=8):
    DH = 64              # head dim
    KD = D // P          # hidden-dim 128-chunks (8)
    NQ = TOK // 512      # 512-wide q chunks (4)
    NT = TOK // P        # 128-wide token chunks (16)
    DC = NH * DH         # per-core head dims (256)
    MQ = DC // P         # 128-row chunks of qT/kT/oT (2)
    HPC = P // DH        # heads per 128-row chunk (2)
    G = NT // 2          # k-chunk pairs per head (8)

    nc = bacc.Bacc("TRN2", target_bir_lowering=False, debug=False,
                   num_devices=num_devices)

    xT_d = nc.declare_dram_parameter("xT", [D, TOK], bf16, isOutput=False)
    wq_d = nc.declare_dram_parameter("wq", [D, DC], bf16, isOutput=False)
    wk_d = nc.declare_dram_parameter("wk", [D, DC], bf16, isOutput=False)
    wv_d = nc.declare_dram_parameter("wv", [D, DC], bf16, isOutput=False)
    wo_d = nc.declare_dram_parameter("wo", [DC, D], bf16, isOutput=False)
    bq_d = nc.declare_dram_parameter("bq", [P, MQ], f32, isOutput=False)
    bk_d = nc.declare_dram_parameter("bk", [P, MQ], f32, isOutput=False)
    bv_d = nc.declare_dram_parameter("bv", [P, DC], f32, isOutput=False)
    id_d = nc.declare_dram_parameter("ident", [P, P], bf16, isOutput=False)
    out_d = nc.declare_dram_parameter("out", [TOK, D], bf16, isOutput=True)

    with tile.TileContext(nc) as tc, ExitStack() as ctx:
        persist = ctx.enter_context(tc.tile_pool(name="persist", bufs=1))
        work = ctx.enter_context(tc.tile_pool(name="work", bufs=1))
        psc = ctx.enter_context(tc.tile_pool(name="psc", bufs=2, space="PSUM"))
        pav = ctx.enter_context(tc.tile_pool(name="pav", bufs=2, space="PSUM"))
        pacc = ctx.enter_context(tc.tile_pool(name="pacc", bufs=2, space="PSUM"))

        # ---- PE warmup: ramp the p-state while the first DMAs land -----
        warm = persist.tile([P, P], f32)
        nc.vector.memset(warm[:], 0.0)
        warm_ps = pacc.tile([P, 512], f32, tag="acc", name="warm_ps")
        for _ in range(13):
            nc.tensor.matmul(warm_ps[:, 0:P], warm[:], warm[:],
                             start=True, stop=True)

        zbias = persist.tile([P, 1], f32)
        nc.vector.memset(zbias[:], 0.0)

        # ---- phase A: DMAs, spread over 4 queues in first-use order ----
        # The SP/ACT sequencers hold each DMA ~1.26us and Pool ~1.05us, so
        # the 32 xT chunks are spread over all three queues in need order.
        wk_sb = work.tile([P, KD, DC], bf16)
        nc.sync.dma_start(wk_sb[:], wk_d.rearrange("(ko ki) n -> ki ko n", ki=P))
        xT_sb = persist.tile([P, KD, TOK], bf16)

        def xchunk(q, n, ko):
            q.dma_start(xT_sb[:, ko, n * 512:(n + 1) * 512],
                        xT_d[ko * P:(ko + 1) * P, n * 512:(n + 1) * 512])

        wq_sb = persist.tile([P, KD, DC], bf16)
        bq_sb = persist.tile([P, MQ], f32)
        bk_sb = persist.tile([P, MQ], f32)
        nc.scalar.dma_start(bq_sb[:], bq_d[:])
        nc.scalar.dma_start(bk_sb[:], bk_d[:])
        wv_sb = work.tile([P, KD, DC], bf16)
        bv_sb = work.tile([P, DC], f32)
        ident = persist.tile([P, P], bf16)
        wo_sb = persist.tile([P, MQ, D], bf16)

        for ko in range(3):                      # SP: wk, n0 ko0-2, wq
            xchunk(nc.sync, 0, ko)
        nc.sync.dma_start(wq_sb[:], wq_d.rearrange("(ko ki) n -> ki ko n", ki=P))
        for ko in range(3, KD):                  # Pool: n0 ko3-7 first
            xchunk(nc.gpsimd, 0, ko)
        nc.gpsimd.dma_start(wv_sb[:], wv_d.rearrange("(ko ki) n -> ki ko n", ki=P))
        nc.gpsimd.dma_start(bv_sb[:], bv_d[:])
        for n in range(1, NQ):
            for ko in range(3):
                xchunk(nc.sync, n, ko)
        for ko in range(3, KD):
            xchunk(nc.gpsimd, 1, ko)
        for ko in range(3, 6):
            xchunk(nc.gpsimd, 2, ko)
        nc.gpsimd.dma_start(ident[:], id_d[:])
        for ko in range(3, 6):
            xchunk(nc.gpsimd, 3, ko)
        nc.gpsimd.dma_start(wo_sb[:], wo_d.rearrange("(mo mi) n -> mi mo n", mi=P))
        for n in (2, 3):                         # ACT: late chunks after wq
            for ko in (6, 7):
                xchunk(nc.scalar, n, ko)

        qT_sb = persist.tile([P, MQ, TOK], f32r)
        kT_sb = persist.tile([P, MQ, TOK], f32r)
        v_sb = persist.tile([P, NT, NH, DH + 1], bf16)
        oT_sb = persist.tile([P, MQ, TOK], bf16)
        onesc = persist.tile([P, 1], f32)
        nc.vector.memset(onesc[:], 1.0)
        nc.vector.tensor_copy(
            v_sb[:, :, :, DH:DH + 1],
            onesc[:, None, :].to_broadcast([P, NT, NH, 1]))

        # preload the Exp table while ACT is idle (zbias: no const-DMA dep)
        wtab = work.tile([1, 1], f32)
        nc.vector.memset(wtab[:], 0.0)
        wtab2 = work.tile([1, 1], f32)
        nc.scalar.activation(wtab2[:], wtab[:],
                             mybir.ActivationFunctionType.Exp,
                             bias=zbias[0:1, :], scale=0.125)

        def proj_block(w_sb, b_sb, t_sb, m, n):
            ps = pacc.tile([P, 512], f32, tag="acc", name="ps")
            for ko in range(KD):
                nc.tensor.matmul(
                    ps[:], w_sb[:, ko, m * P:(m + 1) * P],
                    xT_sb[:, ko, n * 512:(n + 1) * 512],
                    start=(ko == 0), stop=(ko == KD - 1))
            nc.vector.tensor_tensor(
                t_sb[:, m, n * 512:(n + 1) * 512], ps[:],
                b_sb[:, m:m + 1].to_broadcast([P, 512]),
                mybir.AluOpType.add)

        def v_block(t):
            ps = pacc.tile([P, DC], f32, tag="acc", name="vps")
            for ko in range(KD):
                nc.tensor.matmul(
                    ps[:], xT_sb[:, ko, t * P:(t + 1) * P], wv_sb[:, ko, :],
                    start=(ko == 0), stop=(ko == KD - 1))
            nc.vector.tensor_tensor(
                v_sb[:, t, :, 0:DH],
                ps.rearrange("p (h d) -> p h d", h=NH),
                bv_sb.rearrange("p (h d) -> p h d", h=NH),
                mybir.AluOpType.add)

        def emit_scores(pair, n, g, scs):
            qs = slice(n * 512, (n + 1) * 512)
            for j in range(2):
                kk = g * 2 + j
                for h in pair:
                    hm = h // HPC
                    hr = (h % HPC) * DH
                    nc.tensor.matmul(
                        scs[h][:, j, :],
                        kT_sb[hr:hr + DH, hm, kk * P:(kk + 1) * P],
                        qT_sb[hr:hr + DH, hm, qs],
                        start=True, stop=True)

        def emit_av(pair, g, avs, exs):
            # start=False always: the av bank is pre-zeroed by DVE memset, so
            # the four per-qsub accumulation regions in one bank never issue a
            # bank-wide zero (HW start flag marks the whole 2KB zero region).
            for h in pair:
                for j in range(2):
                    kk = g * 2 + j
                    for q in range(4):
                        nc.tensor.matmul(
                            avs[h][:, q, :],
                            exs[h][:, j, q * P:(q + 1) * P],
                            v_sb[:, kk, h, :],
                            start=False,
                            stop=(g == G - 1 and j == 1),
                            skip_group_check=True)

        def emit_normalize(pair, avs, opair):
            for h in pair:
                hr = (h % HPC) * DH
                rec = work.tile([P, NH, 1], f32, tag=f"rec{h % HPC}", bufs=2,
                                name="rec")
                nc.vector.reciprocal(rec[:], avs[h][:, :, DH:DH + 1])
                nc.vector.tensor_tensor(
                    opair[:, :, hr:hr + DH], avs[h][:, :, 0:DH],
                    rec.to_broadcast([P, NH, DH]),
                    mybir.AluOpType.mult)

        def emit_transpose(pi, n, q, opair):
            tp = pav.tile([P, P], bf16, tag="av", name="tp")
            nc.tensor.transpose(tp[:], opair[:, q, :], ident[:])
            nc.vector.tensor_copy(
                oT_sb[:, pi, n * 512 + q * P:n * 512 + (q + 1) * P], tp[:])

        _ou_state = {}  # tok -> (ou tile, halves done)

        def oproj_tile(n, t, nn, act_copy=False):
            tok = n * 4 + t
            ns = slice(nn * 512, (nn + 1) * 512)
            op = pacc.tile([P, 512], f32, tag="acc", name="op")
            for m in range(MQ):
                nc.tensor.matmul(
                    op[:], oT_sb[:, m, tok * P:(tok + 1) * P],
                    wo_sb[:, m, ns],
                    start=(m == 0), stop=(m == MQ - 1))
            if tok not in _ou_state:
                _ou_state[tok] = [work.tile([P, D], bf16, tag="out", bufs=3,
                                            name="ou"), 0]
            ou, done = _ou_state[tok]
            if act_copy:
                nc.scalar.copy(ou[:, ns], op[:])
            else:
                nc.vector.tensor_copy(ou[:, ns], op[:])
            _ou_state[tok][1] = done + 1
            if _ou_state[tok][1] == 2:
                q = (nc.sync, nc.gpsimd, nc.scalar)[tok % 3]
                q.dma_start(out_d[tok * P:(tok + 1) * P, :], ou[:])
                del _ou_state[tok]

        # ---- phase B front: first-scores critical path only ------------
        proj_block(wk_sb, bk_sb, kT_sb, 0, 0)
        proj_block(wq_sb, bq_sb, qT_sb, 0, 0)

        # ---- windowed pipeline: 8 windows = (pair0 n0..3, pair1 n0..3).
        # Window w runs scores+exp for its own (pi, n) while the PE drains
        # the AV matmuls of window w-1 (the exp tiles of w-1 are all ready,
        # so AV never head-of-line-blocks the queue).  Windows 5..7 drain at
        # two g-groups per slot, normalize mid-window, and do transposes +
        # output projection in-window so the tail stays short.
        W = [(pi, n) for pi in range(2) for n in range(NQ)]
        F = [[[] for _ in range(G)] for _ in range(8)]

        def add(w, g, fn, *a):
            F[w][g].append((fn, a))

        # window 0: kT m0 rest (block b by slot 2b), v0..v5, qT m0 n1
        add(0, 0, proj_block, wk_sb, bk_sb, kT_sb, 0, 1)
        add(0, 0, v_block, 0)
        add(0, 1, v_block, 1)
        add(0, 1, v_block, 2)
        add(0, 2, proj_block, wk_sb, bk_sb, kT_sb, 0, 2)
        add(0, 3, v_block, 3)
        add(0, 3, v_block, 4)
        add(0, 4, proj_block, wk_sb, bk_sb, kT_sb, 0, 3)
        add(0, 5, v_block, 5)
        add(0, 6, proj_block, wq_sb, bq_sb, qT_sb, 0, 1)
        # window 1: v6..v15 2/slot (AV(w0, g) at slot g reads v(2g..2g+1);
        # v(2g+1) lands 2+ slots ahead), qT m0 n2
        for i in range(5):
            add(1, i, v_block, 6 + 2 * i)
            add(1, i, v_block, 7 + 2 * i)
        add(1, 6, proj_block, wq_sb, bq_sb, qT_sb, 0, 2)
        # window 2: kT m1 b0/b1, qT m0 n3
        add(2, 1, proj_block, wk_sb, bk_sb, kT_sb, 1, 0)
        add(2, 3, proj_block, wk_sb, bk_sb, kT_sb, 1, 1)
        add(2, 5, proj_block, wq_sb, bq_sb, qT_sb, 0, 3)
        # window 3: kT m1 b2/b3, qT m1 n0
        add(3, 1, proj_block, wk_sb, bk_sb, kT_sb, 1, 2)
        add(3, 3, proj_block, wk_sb, bk_sb, kT_sb, 1, 3)
        add(3, 5, proj_block, wq_sb, bq_sb, qT_sb, 1, 0)
        # windows 4/5: qT m1 rest
        add(4, 1, proj_block, wq_sb, bq_sb, qT_sb, 1, 1)
        add(4, 3, proj_block, wq_sb, bq_sb, qT_sb, 1, 2)
        add(5, 1, proj_block, wq_sb, bq_sb, qT_sb, 1, 3)
        # accel windows: oproj(n) once tp(p1,n) lands at slot 4
        for w, n_o in ((5, 0), (6, 1), (7, 2)):
            for i, (t, nn) in enumerate((t, nn) for t in range(4)
                                        for nn in range(2)):
                add(w, 5 + i // 3, oproj_tile, n_o, t, nn)

        def alloc_avs(pair):
            avs = {h: pav.tile([P, NH, DH + 1], f32, tag="av",
                               name=f"av{h}") for h in pair}
            for h in pair:
                nc.vector.memset(avs[h][:], 0.0)
            return avs

        def normalize_stream(st):
            pair, avs = st["pair"], st["avs"]
            opair = work.tile([P, 4, P], bf16, tag="opair", bufs=2,
                              name="opair")
            emit_normalize(pair, avs, opair)
            return (st["pi"], st["n"], opair)

        def drain_group(st, g):
            emit_av(st["pair"], g, st["avs"], st["exs"][g])

        pend_tp = []        # transposes to emit at the next window's g0
        prev_st = None      # stream of window w-1 awaiting AV
        for w in range(8):
            pi, n = W[w]
            pair = [pi * HPC + i for i in range(HPC)]
            cur_st = {"pi": pi, "n": n, "pair": pair, "exs": []}
            accel = (w >= 5)
            av_cur = None
            for g in range(G):
                scs = {h: psc.tile([P, 2, 512], f32, tag="sc",
                                   name=f"sc{h}") for h in pair}
                emit_scores(pair, n, g, scs)
                exs = {}
                for h in pair:
                    ex = work.tile([P, 2, 512], bf16, tag=f"ex{h % HPC}",
                                   bufs=13, name="ex")
                    nc.scalar.activation(
                        ex[:], scs[h][:],
                        mybir.ActivationFunctionType.Exp,
                        bias=zbias[:, :], scale=0.125)
                    exs[h] = ex
                cur_st["exs"].append(exs)
                if g == 0:
                    for src_pi, src_n, src_op in pend_tp:
                        for q in range(4):
                            emit_transpose(src_pi, src_n, q, src_op)
                    pend_tp = []
                    if prev_st is not None:
                        prev_st["avs"] = alloc_avs(prev_st["pair"])
                for fn, a in F[w][g]:
                    fn(*a)
                if prev_st is not None:
                    if accel:
                        if g < 4:
                            drain_group(prev_st, 2 * g)
                            drain_group(prev_st, 2 * g + 1)
                        elif g == 4:
                            src_pi, src_n, src_op = normalize_stream(prev_st)
                            for q in range(4):
                                emit_transpose(src_pi, src_n, q, src_op)
                            if w == 7:
                                av_cur = alloc_avs(pair)
                                cur_st["avs"] = av_cur
                        if w == 7 and g >= 4:
                            drain_group(cur_st, 2 * (g - 4))
                            drain_group(cur_st, 2 * (g - 4) + 1)
                    else:
                        drain_group(prev_st, g)
            if w == 7:
                tail_st = cur_st
            elif accel:
                prev_st = cur_st
            else:
                if prev_st is not None:
                    pend_tp.append(normalize_stream(prev_st))
                prev_st = cur_st
        # tail: per-qsub normalize so each transpose/oproj chain launches as
        # soon as its own qsub's multiply lands (instead of after a whole-
        # head normalize); nn0 copies go to the now-idle ACT engine
        st = tail_st
        avs = st["avs"]
        opair = work.tile([P, 4, P], bf16, tag="opair", bufs=2, name="opair")
        recs = {}
        for h in st["pair"]:
            rec = work.tile([P, NH, 1], f32, tag=f"rec{h % HPC}", bufs=2,
                            name="rec")
            nc.vector.reciprocal(rec[:], avs[h][:, :, DH:DH + 1])
            recs[h] = rec
        for q in range(4):
            for h in st["pair"]:
                hr = (h % HPC) * DH
                nc.vector.tensor_tensor(
                    opair[:, q, hr:hr + DH], avs[h][:, q, 0:DH],
                    recs[h][:, q, :].to_broadcast([P, DH]),
                    mybir.AluOpType.mult)
            emit_transpose(st["pi"], st["n"], q, opair)
            oproj_tile(3, q, 0, act_copy=True)
            oproj_tile(3, q, 1)
    return nc


_CACHE = {}
LAST_RESULTS = None


def _get_compiled():
    if "nc" not in _CACHE:
        nc = build_core_program()
        nc.compile()
        _CACHE["nc"] = nc
    return _CACHE["nc"]


def kernel(x, wq, bq, wk, bk, wv, bv, wo, bo):
    global LAST_RESULTS
    import ml_dtypes
    bft = ml_dtypes.bfloat16
    x = np.asarray(x, np.float32)
    wq, bq = np.asarray(wq, np.float32), np.asarray(bq, np.float32)
    wk, bk = np.asarray(wk, np.float32), np.asarray(bk, np.float32)
    wv, bv = np.asarray(wv, np.float32), np.asarray(bv, np.float32)
    wo, bo = np.asarray(wo, np.float32), np.asarray(bo, np.float32)
    B, TOK, D = x.shape          # (2, 2048, 1024)
    NH, DH = 4, 64               # heads per core, head dim
    DC = NH * DH                 # 256
    MQ = DC // P                 # 2

    nc = _get_compiled()

    ident = np.eye(P, dtype=np.float32)  # cast per-core below

    in_maps = []
    for c in range(8):
        b, hg = c // 4, c % 4
        sl = slice(hg * DC, (hg + 1) * DC)
        in_maps.append({
            "xT": np.ascontiguousarray(x[b].T).astype(bft),
            "wq": np.ascontiguousarray(wq[:, sl]).astype(bft),
            "wk": np.ascontiguousarray(wk[:, sl]).astype(bft),
            "wv": np.ascontiguousarray(wv[:, sl]).astype(bft),
            "wo": np.ascontiguousarray(wo[sl, :]).astype(bft),
            "bq": np.ascontiguousarray(bq[sl].reshape(MQ, P).T),
            "bk": np.ascontiguousarray(bk[sl].reshape(MQ, P).T),
            "bv": np.ascontiguousarray(np.tile(bv[None, sl], (P, 1))),
            "ident": ident.astype(bft),
        })

    trace = os.environ.get("KERNEL_TRACE", "0") == "1"
    res = run_bass_kernel_spmd(nc, in_maps, core_ids=list(range(8)),
                               trace=trace)
    LAST_RESULTS = res
    outs = [np.asarray(res.results[c]["out"], dtype=np.float32)
            for c in range(8)]
    y = np.stack([sum(outs[0:4]), sum(outs[4:8])], axis=0) + bo[None, None, :]
    return np.ascontiguousarray(y, dtype=np.float32)


# revision 35
# speedup vs baseline: 1.0022x; 1.0022x over previous
"""Multi-head self-attention TRN2 kernel (16 heads, D=1024, x:[2,2048,1024]).

Sharding: 8 cores = 2 (batch) x 4 (head groups of 4 heads).  Host sums the
4 partials per batch and adds bo (the tensor-parallel all-reduce).

Per-core structure (final):
  inputs x/wq/wk/wv DMA'd as bf16 (halves the startup DMA traffic), wo f32r;
  the 32 xT chunks are spread across the SP/Pool/ACT DMA queues in first-use
  order because each queue's sequencer holds a DMA for ~1.1-1.26us.
  qT/kT = (x_b @ wq/wk + b)^T head-dim-major             [256, 2048] f32r
  v     = x_b @ wv + bv, token-major, ones-augmented      [2048,4,65] bf16
  scores: kT-stationary matmuls [k=128chunk, q=512] f32r PSUM
  exp via ACT (scale=1/8, zero-bias AP, no max subtraction) -> bf16 SBUF
  AV q-stationary: lhsT=exp[128k,128q], rhs=v[128k,65] bf16, out [128q,65]
    (65 moving rows/matmul = half the PE-row cost of a d-major AV),
    accumulated with start=False onto DVE-zeroed PSUM (one bank holds the
    four per-qsub regions; the HW start flag would zero the whole 2KB bank)
  normalize: DVE reciprocal of sums column + broadcast multiply -> opair
  PE transpose (vs identity) -> oT, DVE copies PSUM->SBUF
  out = oT^T @ wo per 128-token chunk, bf16 out (host adds bo, upcasts)

Pipeline: 8 windows = (pair0 n0..n3, pair1 n0..n3).  Window w computes
scores+exp for its own 512-q chunk while the PE drains the AV matmuls of
window w-1 (whose exp tiles are all ready, so AV never head-of-line blocks
the in-order PE queue behind an unsatisfied semaphore).  Projections for
k/q/v interleave as PE filler in the early windows where ACT is warming
up; windows 5..7 drain at two g-groups per slot, normalize mid-window, and
run transposes + output projection in-window to keep the tail short.

TimelineSim: 167.6us (baseline v1: 224.1us).  PE busy ~145us (the floor:
projections 41 + scores 54.6 + AV 27.7 + oproj 13.7 + transposes/warmup),
ACT exp 133us, both near-saturated mid-stream.
"""

import os
import sys
from contextlib import ExitStack

import numpy as np

for _p in ("/opt/trn_rl_repo", os.path.expanduser("~/.axon_site/_ro/trn_rl_repo")):
    if os.path.isdir(_p) and _p not in sys.path:
        sys.path.insert(0, _p)

import concourse.bass as bass  # noqa: E402
import concourse.mybir as mybir  # noqa: E402
import concourse.tile as tile  # noqa: E402
from concourse import bacc  # noqa: E402
from concourse.bass_utils import run_bass_kernel_spmd  # noqa: E402

f32 = mybir.dt.float32
f32r = mybir.dt.float32r
bf16 = mybir.dt.bfloat16
P = 128


def build_core_program(D=1024, TOK=2048, NH=4, num_devices=8):
    DH = 64              # head dim
    KD = D // P          # hidden-dim 128-chunks (8)
    NQ = TOK // 512      # 512-wide q chunks (4)
    NT = TOK // P        # 128-wide token chunks (16)
    DC = NH * DH         # per-core head dims (256)
    MQ = DC // P         # 128-row chunks of qT/kT/oT (2)
    HPC = P // DH        # heads per 128-row chunk (2)
    G = NT // 2          # k-chunk pairs per head (8)

    nc = bacc.Bacc("TRN2", target_bir_lowering=False, debug=False,
                   num_devices=num_devices)

    xT_d = nc.declare_dram_parameter("xT", [D, TOK], bf16, isOutput=False)
    wq_d = nc.declare_dram_parameter("wq", [D, DC], bf16, isOutput=False)
    wk_d = nc.declare_dram_parameter("wk", [D, DC], bf16, isOutput=False)
    wv_d = nc.declare_dram_parameter("wv", [D, DC], bf16, isOutput=False)
    wo_d = nc.declare_dram_parameter("wo", [DC, D], f32r, isOutput=False)
    bq_d = nc.declare_dram_parameter("bq", [P, MQ], f32, isOutput=False)
    bk_d = nc.declare_dram_parameter("bk", [P, MQ], f32, isOutput=False)
    bv_d = nc.declare_dram_parameter("bv", [P, DC], f32, isOutput=False)
    id_d = nc.declare_dram_parameter("ident", [P, P], f32r, isOutput=False)
    out_d = nc.declare_dram_parameter("out", [TOK, D], bf16, isOutput=True)

    with tile.TileContext(nc) as tc, ExitStack() as ctx:
        persist = ctx.enter_context(tc.tile_pool(name="persist", bufs=1))
        work = ctx.enter_context(tc.tile_pool(name="work", bufs=1))
        psc = ctx.enter_context(tc.tile_pool(name="psc", bufs=2, space="PSUM"))
        pav = ctx.enter_context(tc.tile_pool(name="pav", bufs=2, space="PSUM"))
        pacc = ctx.enter_context(tc.tile_pool(name="pacc", bufs=2, space="PSUM"))

        # ---- PE warmup: ramp the p-state while the first DMAs land -----
        warm = persist.tile([P, P], f32)
        nc.vector.memset(warm[:], 0.0)
        warm_ps = pacc.tile([P, 512], f32, tag="acc", name="warm_ps")
        for _ in range(13):
            nc.tensor.matmul(warm_ps[:, 0:P], warm[:], warm[:],
                             start=True, stop=True)

        zbias = persist.tile([P, 1], f32)
        nc.vector.memset(zbias[:], 0.0)

        # ---- phase A: DMAs, spread over 4 queues in first-use order ----
        # The SP/ACT sequencers hold each DMA ~1.26us and Pool ~1.05us, so
        # the 32 xT chunks are spread over all three queues in need order.
        wk_sb = work.tile([P, KD, DC], bf16)
        nc.sync.dma_start(wk_sb[:], wk_d.rearrange("(ko ki) n -> ki ko n", ki=P))
        xT_sb = persist.tile([P, KD, TOK], bf16)

        def xchunk(q, n, ko):
            q.dma_start(xT_sb[:, ko, n * 512:(n + 1) * 512],
                        xT_d[ko * P:(ko + 1) * P, n * 512:(n + 1) * 512])

        wq_sb = persist.tile([P, KD, DC], bf16)
        bq_sb = persist.tile([P, MQ], f32)
        bk_sb = persist.tile([P, MQ], f32)
        nc.scalar.dma_start(bq_sb[:], bq_d[:])
        nc.scalar.dma_start(bk_sb[:], bk_d[:])
        wv_sb = work.tile([P, KD, DC], bf16)
        bv_sb = work.tile([P, DC], f32)
        ident = persist.tile([P, P], f32r)
        wo_sb = persist.tile([P, MQ, D], f32r)

        for ko in range(3):                      # SP: wk, n0 ko0-2, wq
            xchunk(nc.sync, 0, ko)
        nc.sync.dma_start(wq_sb[:], wq_d.rearrange("(ko ki) n -> ki ko n", ki=P))
        for ko in range(3, KD):                  # Pool: n0 ko3-7 first
            xchunk(nc.gpsimd, 0, ko)
        nc.gpsimd.dma_start(wv_sb[:], wv_d.rearrange("(ko ki) n -> ki ko n", ki=P))
        nc.gpsimd.dma_start(bv_sb[:], bv_d[:])
        for n in range(1, NQ):
            for ko in range(3):
                xchunk(nc.sync, n, ko)
        for ko in range(3, KD):
            xchunk(nc.gpsimd, 1, ko)
        for ko in range(3, 6):
            xchunk(nc.gpsimd, 2, ko)
        nc.gpsimd.dma_start(ident[:], id_d[:])
        for ko in range(3, 6):
            xchunk(nc.gpsimd, 3, ko)
        nc.gpsimd.dma_start(wo_sb[:], wo_d.rearrange("(mo mi) n -> mi mo n", mi=P))
        for n in (2, 3):                         # ACT: late chunks after wq
            for ko in (6, 7):
                xchunk(nc.scalar, n, ko)

        qT_sb = persist.tile([P, MQ, TOK], f32r)
        kT_sb = persist.tile([P, MQ, TOK], f32r)
        v_sb = persist.tile([P, NT, NH, DH + 1], bf16)
        oT_sb = persist.tile([P, MQ, TOK], f32r)
        onesc = persist.tile([P, 1], f32)
        nc.vector.memset(onesc[:], 1.0)
        nc.vector.tensor_copy(
            v_sb[:, :, :, DH:DH + 1],
            onesc[:, None, :].to_broadcast([P, NT, NH, 1]))

        # preload the Exp table while ACT is idle (zbias: no const-DMA dep)
        wtab = work.tile([1, 1], f32)
        nc.vector.memset(wtab[:], 0.0)
        wtab2 = work.tile([1, 1], f32)
        nc.scalar.activation(wtab2[:], wtab[:],
                             mybir.ActivationFunctionType.Exp,
                             bias=zbias[0:1, :], scale=0.125)

        def proj_block(w_sb, b_sb, t_sb, m, n):
            ps = pacc.tile([P, 512], f32, tag="acc", name="ps")
            for ko in range(KD):
                nc.tensor.matmul(
                    ps[:], w_sb[:, ko, m * P:(m + 1) * P],
                    xT_sb[:, ko, n * 512:(n + 1) * 512],
                    start=(ko == 0), stop=(ko == KD - 1))
            nc.vector.tensor_tensor(
                t_sb[:, m, n * 512:(n + 1) * 512], ps[:],
                b_sb[:, m:m + 1].to_broadcast([P, 512]),
                mybir.AluOpType.add)

        def v_block(t):
            ps = pacc.tile([P, DC], f32, tag="acc", name="vps")
            for ko in range(KD):
                nc.tensor.matmul(
                    ps[:], xT_sb[:, ko, t * P:(t + 1) * P], wv_sb[:, ko, :],
                    start=(ko == 0), stop=(ko == KD - 1))
            nc.vector.tensor_tensor(
                v_sb[:, t, :, 0:DH],
                ps.rearrange("p (h d) -> p h d", h=NH),
                bv_sb.rearrange("p (h d) -> p h d", h=NH),
                mybir.AluOpType.add)

        def emit_scores(pair, n, g, scs):
            qs = slice(n * 512, (n + 1) * 512)
            for j in range(2):
                kk = g * 2 + j
                for h in pair:
                    hm = h // HPC
                    hr = (h % HPC) * DH
                    nc.tensor.matmul(
                        scs[h][:, j, :],
                        kT_sb[hr:hr + DH, hm, kk * P:(kk + 1) * P],
                        qT_sb[hr:hr + DH, hm, qs],
                        start=True, stop=True)

        def emit_av(pair, g, avs, exs):
            # start=False always: the av bank is pre-zeroed by DVE memset, so
            # the four per-qsub accumulation regions in one bank never issue a
            # bank-wide zero (HW start flag marks the whole 2KB zero region).
            for h in pair:
                for j in range(2):
                    kk = g * 2 + j
                    for q in range(4):
                        nc.tensor.matmul(
                            avs[h][:, q, :],
                            exs[h][:, j, q * P:(q + 1) * P],
                            v_sb[:, kk, h, :],
                            start=False,
                            stop=(g == G - 1 and j == 1),
                            skip_group_check=True)

        def emit_normalize(pair, avs, opair):
            for h in pair:
                hr = (h % HPC) * DH
                rec = work.tile([P, NH, 1], f32, tag=f"rec{h % HPC}", bufs=2,
                                name="rec")
                nc.vector.reciprocal(rec[:], avs[h][:, :, DH:DH + 1])
                nc.vector.tensor_tensor(
                    opair[:, :, hr:hr + DH], avs[h][:, :, 0:DH],
                    rec.to_broadcast([P, NH, DH]),
                    mybir.AluOpType.mult)

        def emit_transpose(pi, n, q, opair):
            tp = pav.tile([P, P], f32r, tag="av", name="tp")
            nc.tensor.transpose(tp[:], opair[:, q, :], ident[:])
            nc.vector.tensor_copy(
                oT_sb[:, pi, n * 512 + q * P:n * 512 + (q + 1) * P], tp[:])

        _ou_state = {}  # tok -> (ou tile, halves done)

        def oproj_tile(n, t, nn, act_copy=False):
            tok = n * 4 + t
            ns = slice(nn * 512, (nn + 1) * 512)
            op = pacc.tile([P, 512], f32, tag="acc", name="op")
            for m in range(MQ):
                nc.tensor.matmul(
                    op[:], oT_sb[:, m, tok * P:(tok + 1) * P],
                    wo_sb[:, m, ns],
                    start=(m == 0), stop=(m == MQ - 1))
            if tok not in _ou_state:
                _ou_state[tok] = [work.tile([P, D], bf16, tag="out", bufs=3,
                                            name="ou"), 0]
            ou, done = _ou_state[tok]
            if act_copy:
                nc.scalar.copy(ou[:, ns], op[:])
            else:
                nc.vector.tensor_copy(ou[:, ns], op[:])
            _ou_state[tok][1] = done + 1
            if _ou_state[tok][1] == 2:
                q = (nc.sync, nc.gpsimd, nc.scalar)[tok % 3]
                q.dma_start(out_d[tok * P:(tok + 1) * P, :], ou[:])
                del _ou_state[tok]

        # ---- phase B front: first-scores critical path only ------------
        proj_block(wk_sb, bk_sb, kT_sb, 0, 0)
        proj_block(wq_sb, bq_sb, qT_sb, 0, 0)

        # ---- windowed pipeline: 8 windows = (pair0 n0..3, pair1 n0..3).
        # Window w runs scores+exp for its own (pi, n) while the PE drains
        # the AV matmuls of window w-1 (the exp tiles of w-1 are all ready,
        # so AV never head-of-line-blocks the queue).  Windows 5..7 drain at
        # two g-groups per slot, normalize mid-window, and do transposes +
        # output projection in-window so the tail stays short.
        W = [(pi, n) for pi in range(2) for n in range(NQ)]
        F = [[[] for _ in range(G)] for _ in range(8)]

        def add(w, g, fn, *a):
            F[w][g].append((fn, a))

        # window 0: kT m0 rest (block b by slot 2b), v0..v5, qT m0 n1
        add(0, 0, proj_block, wk_sb, bk_sb, kT_sb, 0, 1)
        add(0, 0, v_block, 0)
        add(0, 1, v_block, 1)
        add(0, 1, v_block, 2)
        add(0, 2, proj_block, wk_sb, bk_sb, kT_sb, 0, 2)
        add(0, 3, v_block, 3)
        add(0, 3, v_block, 4)
        add(0, 4, proj_block, wk_sb, bk_sb, kT_sb, 0, 3)
        add(0, 5, v_block, 5)
        add(0, 6, proj_block, wq_sb, bq_sb, qT_sb, 0, 1)
        # window 1: v6..v15 2/slot (AV(w0, g) at slot g reads v(2g..2g+1);
        # v(2g+1) lands 2+ slots ahead), qT m0 n2
        for i in range(5):
            add(1, i, v_block, 6 + 2 * i)
            add(1, i, v_block, 7 + 2 * i)
        add(1, 6, proj_block, wq_sb, bq_sb, qT_sb, 0, 2)
        # window 2: kT m1 b0/b1, qT m0 n3
        add(2, 1, proj_block, wk_sb, bk_sb, kT_sb, 1, 0)
        add(2, 3, proj_block, wk_sb, bk_sb, kT_sb, 1, 1)
        add(2, 5, proj_block, wq_sb, bq_sb, qT_sb, 0, 3)
        # window 3: kT m1 b2/b3, qT m1 n0
        add(3, 1, proj_block, wk_sb, bk_sb, kT_sb, 1, 2)
        add(3, 3, proj_block, wk_sb, bk_sb, kT_sb, 1, 3)
        add(3, 5, proj_block, wq_sb, bq_sb, qT_sb, 1, 0)
        # windows 4/5: qT m1 rest
        add(4, 1, proj_block, wq_sb, bq_sb, qT_sb, 1, 1)
        add(4, 3, proj_block, wq_sb, bq_sb, qT_sb, 1, 2)
        add(5, 1, proj_block, wq_sb, bq_sb, qT_sb, 1, 3)
        # accel windows: oproj(n) once tp(p1,n) lands at slot 4
        for w, n_o in ((5, 0), (6, 1), (7, 2)):
            for i, (t, nn) in enumerate((t, nn) for t in range(4)
                                        for nn in range(2)):
                add(w, 5 + i // 3, oproj_tile, n_o, t, nn)

        def alloc_avs(pair):
            avs = {h: pav.tile([P, NH, DH + 1], f32, tag="av",
                               name=f"av{h}") for h in pair}
            for h in pair:
                nc.vector.memset(avs[h][:], 0.0)
            return avs

        def normalize_stream(st):
            pair, avs = st["pair"], st["avs"]
            opair = work.tile([P, 4, P], f32r, tag="opair", bufs=2,
                              name="opair")
            emit_normalize(pair, avs, opair)
            return (st["pi"], st["n"], opair)

        def drain_group(st, g):
            emit_av(st["pair"], g, st["avs"], st["exs"][g])

        pend_tp = []        # transposes to emit at the next window's g0
        prev_st = None      # stream of window w-1 awaiting AV
        for w in range(8):
            pi, n = W[w]
            pair = [pi * HPC + i for i in range(HPC)]
            cur_st = {"pi": pi, "n": n, "pair": pair, "exs": []}
            accel = (w >= 5)
            av_cur = None
            for g in range(G):
                scs = {h: psc.tile([P, 2, 512], f32, tag="sc",
                                   name=f"sc{h}") for h in pair}
                emit_scores(pair, n, g, scs)
                exs = {}
                for h in pair:
                    ex = work.tile([P, 2, 512], bf16, tag=f"ex{h % HPC}",
                                   bufs=13, name="ex")
                    nc.scalar.activation(
                        ex[:], scs[h][:],
                        mybir.ActivationFunctionType.Exp,
                        bias=zbias[:, :], scale=0.125)
                    exs[h] = ex
                cur_st["exs"].append(exs)
                if g == 0:
                    for src_pi, src_n, src_op in pend_tp:
                        for q in range(4):
                            emit_transpose(src_pi, src_n, q, src_op)
                    pend_tp = []
                    if prev_st is not None:
                        prev_st["avs"] = alloc_avs(prev_st["pair"])
                for fn, a in F[w][g]:
                    fn(*a)
                if prev_st is not None:
                    if accel:
                        if g < 4:
                            drain_group(prev_st, 2 * g)
                            drain_group(prev_st, 2 * g + 1)
                        elif g == 4:
                            src_pi, src_n, src_op = normalize_stream(prev_st)
                            for q in range(4):
                                emit_transpose(src_pi, src_n, q, src_op)
                            if w == 7:
                                av_cur = alloc_avs(pair)
                                cur_st["avs"] = av_cur
                        if w == 7 and g >= 4:
                            drain_group(cur_st, 2 * (g - 4))
                            drain_group(cur_st, 2 * (g - 4) + 1)
                    else:
                        drain_group(prev_st, g)
            if w == 7:
                tail_st = cur_st
            elif accel:
                prev_st = cur_st
            else:
                if prev_st is not None:
                    pend_tp.append(normalize_stream(prev_st))
                prev_st = cur_st
        # tail: per-qsub normalize so each transpose/oproj chain launches as
        # soon as its own qsub's multiply lands (instead of after a whole-
        # head normalize); nn0 copies go to the now-idle ACT engine
        st = tail_st
        avs = st["avs"]
        opair = work.tile([P, 4, P], f32r, tag="opair", bufs=2, name="opair")
        recs = {}
        for h in st["pair"]:
            rec = work.tile([P, NH, 1], f32, tag=f"rec{h % HPC}", bufs=2,
                            name="rec")
            nc.vector.reciprocal(rec[:], avs[h][:, :, DH:DH + 1])
            recs[h] = rec
        for q in range(4):
            for h in st["pair"]:
                hr = (h % HPC) * DH
                nc.vector.tensor_tensor(
                    opair[:, q, hr:hr + DH], avs[h][:, q, 0:DH],
                    recs[h][:, q, :].to_broadcast([P, DH]),
                    mybir.AluOpType.mult)
            emit_transpose(st["pi"], st["n"], q, opair)
            oproj_tile(3, q, 0, act_copy=True)
            oproj_tile(3, q, 1)
    return nc


_CACHE = {}
LAST_RESULTS = None


def _get_compiled():
    if "nc" not in _CACHE:
        nc = build_core_program()
        nc.compile()
        _CACHE["nc"] = nc
    return _CACHE["nc"]


def kernel(x, wq, bq, wk, bk, wv, bv, wo, bo):
    global LAST_RESULTS
    import ml_dtypes
    bft = ml_dtypes.bfloat16
    x = np.asarray(x, np.float32)
    wq, bq = np.asarray(wq, np.float32), np.asarray(bq, np.float32)
    wk, bk = np.asarray(wk, np.float32), np.asarray(bk, np.float32)
    wv, bv = np.asarray(wv, np.float32), np.asarray(bv, np.float32)
    wo, bo = np.asarray(wo, np.float32), np.asarray(bo, np.float32)
    B, TOK, D = x.shape          # (2, 2048, 1024)
    NH, DH = 4, 64               # heads per core, head dim
    DC = NH * DH                 # 256
    MQ = DC // P                 # 2

    nc = _get_compiled()

    ident = np.eye(P, dtype=np.float32)  # cast per-core below

    in_maps = []
    for c in range(8):
        b, hg = c // 4, c % 4
        sl = slice(hg * DC, (hg + 1) * DC)
        in_maps.append({
            "xT": np.ascontiguousarray(x[b].T).astype(bft),
            "wq": np.ascontiguousarray(wq[:, sl]).astype(bft),
            "wk": np.ascontiguousarray(wk[:, sl]).astype(bft),
            "wv": np.ascontiguousarray(wv[:, sl]).astype(bft),
            "wo": np.ascontiguousarray(wo[sl, :]),
            "bq": np.ascontiguousarray(bq[sl].reshape(MQ, P).T),
            "bk": np.ascontiguousarray(bk[sl].reshape(MQ, P).T),
            "bv": np.ascontiguousarray(np.tile(bv[None, sl], (P, 1))),
            "ident": ident,
        })

    trace = os.environ.get("KERNEL_TRACE", "0") == "1"
    res = run_bass_kernel_spmd(nc, in_maps, core_ids=list(range(8)),
                               trace=trace)
    LAST_RESULTS = res
    outs = [np.asarray(res.results[c]["out"], dtype=np.float32)
            for c in range(8)]
    y = np.stack([sum(outs[0:4]), sum(outs[4:8])], axis=0) + bo[None, None, :]
    return np.ascontiguousarray(y, dtype=np.float32)
